# revision 38
# baseline (speedup 1.0000x reference)
"""Trainium2 Bass kernel for nn_MultiHeadAttention (B=2, S=2048, DM=1024, H=8).

Sharding: data-parallel on batch x tensor-parallel on heads.
Core c in 0..7 handles batch b = c//4 and heads {2*(c%4), 2*(c%4)+1}.

v2 (pipelined): same math as the v1 baseline (feature-major dataflow,
S^T score tiles, rowsum-via-matmul softmax denominators) restructured to
keep the tensor engine streaming:
  - attention inner loop is software-pipelined: scores for oc+2 are issued
    ahead of the PV/rowsum matmuls of oc, so the in-order PE queue never
    head-blocks on the exp->mask chain
  - PSUM evacuations split between the scalar engine (Identity+bias, idle
    outside the exp phase) and the vector engine
  - unit epilogues (reciprocal/normalize) are deferred into the next
    unit's loop to hide the DRAM round-trip latency of the r-chain
  - DMA issue order prioritized: x_q,x_k -> x_v -> mask(half0) -> mask(half1),
    round-robin across sync/gpsimd/vector queues; exp ACT table preloaded
  - out-projection in two windows (after each query-half), bf16 partials
"""

import sys

sys.path.insert(0, "/opt/trn_rl_repo")

import numpy as np
import ml_dtypes

import concourse.bass as bass
import concourse.tile as tile
from concourse import bacc, mybir
from concourse.bass import ts, ds
from concourse.bass_utils import run_bass_kernel_spmd

BF16 = mybir.dt.bfloat16
F32 = mybir.dt.float32
Exp = mybir.ActivationFunctionType.Exp
Ident = mybir.ActivationFunctionType.Identity

B, S, DM, H, DOUT = 2, 2048, 1024, 8, 1024
D = DM // H            # 128 head dim
NH = 2                 # heads per core
KC = DM // 128         # 8 contraction chunks for projections
OC = S // 128          # 16 key chunks
NT = 512               # PSUM-bank-sized free tile (fp32)
HQ = 1024              # queries per half
SCALE = float(1.0 / np.sqrt(np.float32(D)))


def build():
    nc = bacc.Bacc(None, target_bir_lowering=False)

    xT = nc.dram_tensor("xT", [3, DM, S], BF16, kind="ExternalInput")
    maskT = nc.dram_tensor("maskT", [S, S], BF16, kind="ExternalInput")
    w_qkv = nc.dram_tensor("w_qkv", [128, 3, KC, NH, D], BF16, kind="ExternalInput")
    b_qkv = nc.dram_tensor("b_qkv", [128, 3, NH], F32, kind="ExternalInput")
    wo = nc.dram_tensor("wo", [D, NH, DOUT], BF16, kind="ExternalInput")
    ident = nc.dram_tensor("ident", [128, 128], BF16, kind="ExternalInput")
    bo = nc.dram_tensor("bo", [128, DOUT // 128], F32, kind="ExternalInput")
    outT = nc.dram_tensor("outT", [DOUT, S], BF16, kind="ExternalOutput")

    xq3 = [nc.sync, nc.gpsimd, nc.scalar]

    with tile.TileContext(nc) as tc:
        with (
            tc.tile_pool(name="const", bufs=1) as constp,
            tc.tile_pool(name="xin", bufs=17) as xp,
            tc.tile_pool(name="maskp", bufs=24) as mp,
            tc.tile_pool(name="pwork", bufs=3) as pw,
            tc.tile_pool(name="rwork", bufs=2) as rbp,
            tc.tile_pool(name="fout", bufs=4) as fop,
            tc.tile_pool(name="psum", bufs=4, space="PSUM") as psp,
            tc.tile_pool(name="dram", bufs=2, space="DRAM") as dramp,
        ):
            # ---- small constants + exp table preload ----
            ones_col = constp.tile([128, 1], BF16)
            nc.vector.memset(ones_col, 1.0)
            dummy = constp.tile([128, 1], BF16)
            # preload the exp table set so the first real exp doesn't pay it
            nc.scalar.activation(out=dummy, in_=ones_col, func=Exp, bias=0.0, scale=1.0)
            b_sb = constp.tile([128, 3, NH], F32)
            nc.gpsimd.dma_start(out=b_sb, in_=b_qkv[:])

            # ---- weights early on dedicated rings, then q/k chunk halves ----
            # v-chunk DMA *issues* are deferred until after the q/k passes so
            # no engine queue ever head-blocks on an x-slot WAR wait ahead of
            # the PSUM evacuations.
            # w_q split so the first projection matmuls wait only on the k=0,1
            # weight chunks (~131KB) instead of the full 524KB tensor
            w_sb = constp.tile([128, 3, KC, NH, D], BF16)
            nc.scalar.dma_start(out=w_sb[:, 0, 0:2, :, :], in_=w_qkv[:, 0, 0:2, :, :])
            ident_sb = constp.tile([128, 128], BF16)
            xts = []  # [t][k]
            for t in range(3):
                xts.append(
                    [xp.tile([128, S], BF16, tag="x", name=f"x{t}_{k}") for k in range(KC)]
                )
            # first q chunk in quarter-slices so pass A's first matmul fires asap
            nc.sync.dma_start(out=xts[0][0][:, ts(0, NT)], in_=xT[0, 0:128, ts(0, NT)])
            nc.sync.dma_start(out=xts[0][0][:, ds(NT, NT)], in_=xT[0, 0:128, ds(NT, NT)])
            nc.scalar.dma_start(
                out=w_sb[:, 0, 2:KC, :, :], in_=w_qkv[:, 0, 2:KC, :, :]
            )
            qi = 1
            for t in range(2):
                for cb in range(2):
                    for k in range(KC):
                        if t == 0 and cb == 0 and k == 0:
                            continue  # issued above
                        xq3[qi % 3].dma_start(
                            out=xts[t][k][:, ts(cb, HQ)],
                            in_=xT[t, k * 128 : (k + 1) * 128, ts(cb, HQ)],
                        )
                        qi += 1
                    if t == 0 and cb == 0:
                        # q passA chunks are queued on every ring; later-needed
                        # weights ride behind them
                        nc.gpsimd.dma_start(
                            out=w_sb[:, 1, :, :, :], in_=w_qkv[:, 1, :, :, :]
                        )
                        nc.scalar.dma_start(out=ident_sb, in_=ident[:])
                    if t == 1 and cb == 0:
                        nc.gpsimd.dma_start(
                            out=w_sb[:, 2, :, :, :], in_=w_qkv[:, 2, :, :, :]
                        )

            mtile = {
                (half, oc): mp.tile([128, HQ], BF16, tag="mask", name=f"m{half}_{oc}")
                for half in range(2)
                for oc in range(OC)
            }
            bo_sb = constp.tile([128, DOUT // 128], F32)
            wo_sb = constp.tile([D, NH, DOUT], BF16)

            def issue_late_dmas():
                # called after the q/k projection passes are issued: v chunks,
                # then masks, then out-projection constants
                xq2 = [nc.sync, nc.gpsimd]
                n = 0
                for cb in range(2):
                    for k in range(KC):
                        xq2[n % 2].dma_start(
                            out=xts[2][k][:, ts(cb, HQ)],
                            in_=xT[2, k * 128 : (k + 1) * 128, ts(cb, HQ)],
                        )
                        n += 1
                # mask half0 on sync, half1 on gpsimd so each unit's r-chain
                # finds at least one drained ring
                for half in range(2):
                    for oc in range(OC):
                        xq2[half].dma_start(
                            out=mtile[(half, oc)],
                            in_=maskT[
                                oc * 128 : (oc + 1) * 128, half * HQ : (half + 1) * HQ
                            ],
                        )
                    if half == 0:
                        nc.gpsimd.dma_start(out=bo_sb, in_=bo[:])
                        nc.gpsimd.dma_start(out=wo_sb, in_=wo[:])

            qk_sb = constp.tile([128, 2, NH, S], BF16)
            vpt_sb = constp.tile([128, NH, S], BF16)
            vp_sb = constp.tile([128, OC, NH, D], BF16)
            outn_sb = constp.tile([128, NH, S], BF16)

            def evac_bias(n, dst, src, bias_ap):
                # alternate scalar/vector engines for PSUM evacuation
                if n % 2 == 0:
                    nc.scalar.activation(out=dst, in_=src, func=Ident, bias=bias_ap, scale=1.0)
                else:
                    nc.vector.tensor_scalar_add(out=dst, in0=src, scalar1=bias_ap)

            # ---- projections: two passes per tensor, chasing the DMA stream.
            # pass cb covers query-columns [cb*1024, cb*1024+1024) for BOTH
            # heads so each arriving x half-chunk is consumed immediately.
            def proj_pass(t, cb, dst):
                acc = {}
                for h in range(NH):
                    for i2 in range(2):
                        acc[(h, i2)] = psp.tile(
                            [128, NT], F32, tag="acc", name=f"acc{t}_{cb}_{h}_{i2}"
                        )
                for k in range(KC):
                    for h in range(NH):
                        for i2 in range(2):
                            nc.tensor.matmul(
                                acc[(h, i2)],
                                w_sb[:, t, k, h, :],
                                xts[t][k][:, ds(cb * HQ + i2 * NT, NT)],
                                start=(k == 0),
                                stop=(k == KC - 1),
                            )
                for n, h in enumerate(range(NH)):
                    for i2 in range(2):
                        evac_bias(
                            n * 2 + i2,
                            dst[h][:, ds(cb * HQ + i2 * NT, NT)],
                            acc[(h, i2)],
                            b_sb[:, t, h : h + 1],
                        )

            for t in range(2):
                for cb in range(2):
                    proj_pass(t, cb, [qk_sb[:, t, h, :] for h in range(NH)])

            issue_late_dmas()

            # ---- V projection + PE transpose to natural layout ----
            def v_transposes(cb):
                # key chunks oc in [cb*8, cb*8+8) for both heads
                for h in range(NH):
                    for oc in range(cb * 8, cb * 8 + 8):
                        tps = psp.tile(
                            [128, D], BF16, tag="s", bufs=2, name=f"tps{h}_{oc}"
                        )
                        nc.tensor.transpose(
                            tps, vpt_sb[:, h, ds(oc * 128, 128)], ident_sb
                        )
                        nc.vector.tensor_copy(vp_sb[:, oc, h, :], tps)

            # ---- attention units: (half, head), pipelined inner loop ----
            pending = []  # deferred DVE epilogue thunks from the previous unit

            def make_se(u, half, hh):
                i0 = half * HQ
                sps = {}
                pms = {}

                def scores(oc):
                    t_ = psp.tile([128, 2 * NT], F32, tag="s", bufs=2, name=f"sps{u}_{oc}")
                    for j in range(2):
                        nc.tensor.matmul(
                            t_[:, ts(j, NT)],
                            qk_sb[:, 1, hh, ds(oc * 128, 128)],
                            qk_sb[:, 0, hh, ds(i0 + j * NT, NT)],
                            start=True,
                            stop=True,
                        )
                    sps[oc] = t_

                def expmul(oc):
                    p = pw.tile([128, 2 * NT], BF16, tag="p", name=f"p{u}_{oc}")
                    nc.scalar.activation(
                        out=p, in_=sps.pop(oc), func=Exp, bias=0.0, scale=SCALE
                    )
                    pm = pw.tile([128, 2 * NT], BF16, tag="pm", name=f"pm{u}_{oc}")
                    nc.vector.tensor_mul(pm, p, mtile[(half, oc)])
                    pms[oc] = pm

                return scores, expmul, sps, pms


            # hoist u0's first scores + exps ahead of the V phase (PE is about
            # to wait on v-chunk DMAs; ACT is idle) — muls wait for mask in u0
            u0_state = make_se(0, 0, 0)
            u0_ps = {}
            u0_state[0](0)
            u0_state[0](1)
            for oc in (0, 1):
                p_ = pw.tile([128, 2 * NT], BF16, tag="p", name=f"p0h_{oc}")
                nc.scalar.activation(
                    out=p_, in_=u0_state[2].pop(oc), func=Exp, bias=0.0, scale=SCALE
                )
                u0_ps[oc] = p_

            proj_pass(2, 0, [vpt_sb[:, h, :] for h in range(NH)])
            proj_pass(2, 1, [vpt_sb[:, h, :] for h in range(NH)])
            v_transposes(0)
            v_transposes(1)

            def do_unit(u, half, hh):
                i0 = half * HQ
                outp = [
                    psp.tile([128, NT], F32, tag="acc", name=f"outp{u}_{j}")
                    for j in range(2)
                ]
                rp = psp.tile([128, NT], F32, tag="acc", name=f"rp{u}")
                if u != 0:
                    scores, expmul, sps, pms = make_se(u, half, hh)

                if u == 0:
                    # scores+exp for oc 0,1 were issued before the V phase;
                    # finish their mask-muls now (mask has long arrived)
                    scores, expmul, sps, pms = u0_state
                    for oc in (0, 1):
                        pm = pw.tile([128, 2 * NT], BF16, tag="pm", name=f"pm0_{oc}")
                        nc.vector.tensor_mul(pm, u0_ps.pop(oc), mtile[(0, oc)])
                        pms[oc] = pm
                else:
                    scores(0)
                    expmul(0)
                    scores(1)
                    expmul(1)
                for oc in range(OC):
                    # drain one deferred thunk from the previous unit's epilogue
                    if oc in (3, 5, 7) and pending:
                        pending.pop(0)()
                    if oc + 2 < OC:
                        scores(oc + 2)
                    pm = pms.pop(oc)
                    for j in range(2):
                        nc.tensor.matmul(
                            outp[j],
                            vp_sb[:, oc, hh, :],
                            pm[:, ts(j, NT)],
                            start=(oc == 0),
                            stop=(oc == OC - 1),
                        )
                    for j in range(2):
                        nc.tensor.matmul(
                            rp[32 * j : 32 * j + 1, :],
                            ones_col,
                            pm[:, ts(j, NT)],
                            start=(oc == 0),
                            stop=(oc == OC - 1),
                        )
                    if oc + 2 < OC:
                        expmul(oc + 2)

                # epilogue: evacuate rowsums first (critical path), then outp;
                # two per-j r-chains run in parallel on separate queues.
                # reciprocal + normalize deferred (returned as thunks)
                rq = [nc.gpsimd, nc.sync]
                r2 = rbp.tile([33, NT], F32, tag="r2", name=f"r2_{u}")
                for j in range(2):
                    nc.vector.tensor_copy(
                        r2[32 * j : 32 * j + 1, :], rp[32 * j : 32 * j + 1, :]
                    )
                rd = dramp.tile([2, NT], F32, tag="rd", name=f"rd{u}")
                for j in range(2):
                    rq[j].dma_start(out=rd[j : j + 1, :], in_=r2[32 * j : 32 * j + 1, :])
                rsegs = [
                    rbp.tile([64, 8], F32, tag=f"rseg{j}", name=f"rseg{u}_{j}")
                    for j in range(2)
                ]
                for j in range(2):
                    rq[j].dma_start(
                        out=rsegs[j],
                        in_=rd[j : j + 1, :]
                        .rearrange("a b -> (a b)")
                        .rearrange("(p c) -> p c", p=64),
                    )
                osb = rbp.tile([128, 2 * NT], F32, tag="osb", name=f"osb{u}")
                nc.scalar.copy(osb[:, ts(0, NT)], outp[0])
                nc.vector.tensor_copy(osb[:, ts(1, NT)], outp[1])
                rsegs2 = [
                    rbp.tile([64, 8], F32, tag=f"rseg2_{j}", name=f"rseg2_{u}_{j}")
                    for j in range(2)
                ]
                rd2 = dramp.tile([2, NT], F32, tag="rd2", name=f"rd2_{u}")
                rbcs = [
                    rbp.tile([128, NT], F32, tag="rbc", name=f"rbc{u}_{j}")
                    for j in range(2)
                ]

                def t_recip(j):
                    nc.vector.reciprocal(rsegs2[j], rsegs[j])
                    rq[j].dma_start(
                        out=rd2[j : j + 1, :]
                        .rearrange("a b -> (a b)")
                        .rearrange("(p c) -> p c", p=64),
                        in_=rsegs2[j],
                    )
                    rq[j].dma_start(
                        out=rbcs[j], in_=rd2[j : j + 1, :].to_broadcast([128, NT])
                    )

                def t_norm(j):
                    # normalize on the (idle) gpsimd engine: all-SBUF operands,
                    # keeps the vector queue free for the next unit's mask-muls
                    nc.gpsimd.tensor_mul(
                        outn_sb[:, hh, ds(i0 + j * NT, NT)], osb[:, ts(j, NT)], rbcs[j]
                    )

                return [
                    lambda: t_recip(0),
                    lambda: (t_recip(1), t_norm(0)),
                    lambda: t_norm(1),
                ]

            def window(half, lo=0, hi=16):
                # out-projection tiles [lo, hi) for one query half (n = itl*8+dc)
                i0 = half * HQ
                outq = [nc.sync, nc.gpsimd, nc.scalar]
                for n in range(lo, hi):
                    itl, dc = n // 8, n % 8
                    # alternate PSUM tags for a deeper evacuation pipeline
                    if n % 2 == 0:
                        facc = psp.tile(
                            [128, NT], F32, tag="s", bufs=2, name=f"facc{half}_{itl}_{dc}"
                        )
                    else:
                        facc = psp.tile(
                            [128, NT], F32, tag="acc", name=f"facc{half}_{itl}_{dc}"
                        )
                    for hh in range(NH):
                        nc.tensor.matmul(
                            facc,
                            wo_sb[:, hh, ds(dc * 128, 128)],
                            outn_sb[:, hh, ds(i0 + itl * NT, NT)],
                            start=(hh == 0),
                            stop=(hh == NH - 1),
                        )
                    fsb = fop.tile([128, NT], BF16, tag="f", name=f"f{half}_{itl}_{dc}")
                    evac_bias(n, fsb, facc, bo_sb[:, dc : dc + 1])
                    outq[n % 3].dma_start(
                        out=outT[ds(dc * 128, 128), ds(i0 + itl * NT, NT)], in_=fsb
                    )

            # order: u0, u1, u2, most of winA(half0), u3, rest of winA, winB.
            # winA placed after u2 so u1's r-chain latency hides inside u2;
            # winA's last tiles run after u3 to cover u3's r-chain latency.
            pending.extend(do_unit(0, 0, 0))
            pending.extend(do_unit(1, 0, 1))
            pending.extend(do_unit(2, 1, 0))
            window(0, 0, 6)
            pending.extend(do_unit(3, 1, 1))
            # drain u3's r-chain immediately (recip on DVE, norms on gpsimd)
            # so winA's remaining tiles execute while it resolves
            for th in pending:
                th()
            pending.clear()
            window(0, 6, 16)
            window(1)

    return nc


_NC_CACHE = None


def _get_nc():
    global _NC_CACHE
    if _NC_CACHE is None:
        nc = build()
        nc.compile()
        _NC_CACHE = nc
    return _NC_CACHE


def make_in_maps(q, k, v, mask, Wq, bq, Wk, bk, Wv, bv, Wo, bo):
    bf = ml_dtypes.bfloat16
    q = np.asarray(q, np.float32)
    k = np.asarray(k, np.float32)
    v = np.asarray(v, np.float32)
    mask = np.asarray(mask)
    Ws = [np.asarray(w, np.float32) for w in (Wq, Wk, Wv)]
    bs = [np.asarray(b_, np.float32) for b_ in (bq, bk, bv)]
    Wo = np.asarray(Wo, np.float32)
    bo = np.asarray(bo, np.float32)

    xTb, maskTb = [], []
    for b in range(B):
        xTb.append(
            np.ascontiguousarray(np.stack([q[b].T, k[b].T, v[b].T]).astype(bf))
        )
        maskTb.append(
            np.ascontiguousarray(mask[b].T.astype(np.float32)).astype(bf)
        )
    # W[dm, dout] with head h owning columns d*H+h; reshape for tile slicing:
    # Wr[t][kc, p, d, h] = W[kc*128+p, d*H+h]
    Wr = [W.reshape(KC, 128, D, H) for W in Ws]
    br = [b_.reshape(D, H) for b_ in bs]

    ident = np.eye(128, dtype=np.float32).astype(bf)

    in_maps = []
    for c in range(8):
        b = c // 4
        h0 = NH * (c % 4)
        w_core = np.empty((128, 3, KC, NH, D), np.float32)
        for t in range(3):
            for hi in range(NH):
                w_core[:, t, :, hi, :] = Wr[t][:, :, :, h0 + hi].transpose(1, 0, 2)
        b_core = np.empty((128, 3, NH), np.float32)
        for t in range(3):
            for hi in range(NH):
                b_core[:, t, hi] = br[t][:, h0 + hi]
        wo_core = np.stack([Wo[h0 + hi :: H, :] for hi in range(NH)], axis=1)
        bo_core = bo if c % 4 == 0 else np.zeros_like(bo)
        in_maps.append(
            {
                "xT": xTb[b],
                "ident": ident,
                "maskT": maskTb[b],
                "w_qkv": np.ascontiguousarray(w_core).astype(bf),
                "b_qkv": np.ascontiguousarray(b_core),
                "wo": np.ascontiguousarray(wo_core).astype(bf),
                "bo": np.ascontiguousarray(bo_core.reshape(DOUT // 128, 128).T),
            }
        )
    return in_maps


def unshard(results):
    out = np.zeros((B, DOUT, S), np.float32)
    for c in range(8):
        out[c // 4] += np.asarray(results[c]["outT"], np.float32)
    return np.ascontiguousarray(out.transpose(0, 2, 1))


def kernel(**inputs):
    in_maps = make_in_maps(**inputs)
    nc = _get_nc()
    res = run_bass_kernel_spmd(nc, in_maps, core_ids=list(range(8)))
    return unshard(res.results)


# revision 40
# speedup vs baseline: 1.1620x; 1.1620x over previous
"""Trainium2 Bass kernel for nn_MultiHeadAttention (B=2, S=2048, DM=1024, H=8).

Sharding: data-parallel on batch x tensor-parallel on heads.
Core c in 0..7 handles batch b = c//4 and heads {2*(c%4), 2*(c%4)+1}.

v2 (pipelined): same math as the v1 baseline (feature-major dataflow,
S^T score tiles, rowsum-via-matmul softmax denominators) restructured to
keep the tensor engine streaming:
  - attention inner loop is software-pipelined: scores for oc+2 are issued
    ahead of the PV/rowsum matmuls of oc, so the in-order PE queue never
    head-blocks on the exp->mask chain
  - PSUM evacuations split between the scalar engine (Identity+bias, idle
    outside the exp phase) and the vector engine
  - unit epilogues (reciprocal/normalize) are deferred into the next
    unit's loop to hide the DRAM round-trip latency of the r-chain
  - DMA issue order prioritized: x_q,x_k -> x_v -> mask(half0) -> mask(half1),
    round-robin across sync/gpsimd/vector queues; exp ACT table preloaded
  - out-projection in two windows (after each query-half), bf16 partials
"""

import sys

sys.path.insert(0, "/opt/trn_rl_repo")

import numpy as np
import ml_dtypes

import concourse.bass as bass
import concourse.tile as tile
from concourse import bacc, mybir
from concourse.bass import ts, ds
from concourse.bass_utils import run_bass_kernel_spmd

BF16 = mybir.dt.bfloat16
F32 = mybir.dt.float32
Exp = mybir.ActivationFunctionType.Exp
Ident = mybir.ActivationFunctionType.Identity

B, S, DM, H, DOUT = 2, 2048, 1024, 8, 1024
D = DM // H            # 128 head dim
NH = 2                 # heads per core
KC = DM // 128         # 8 contraction chunks for projections
OC = S // 128          # 16 key chunks
NT = 512               # PSUM-bank-sized free tile (fp32)
HQ = 1024              # queries per half
SCALE = float(1.0 / np.sqrt(np.float32(D)))


def build():
    nc = bacc.Bacc(None, target_bir_lowering=False)

    xT = nc.dram_tensor("xT", [3, DM, S], BF16, kind="ExternalInput")
    maskT = nc.dram_tensor("maskT", [S, S], BF16, kind="ExternalInput")
    w_qkv = nc.dram_tensor("w_qkv", [128, 3, KC, NH, D], BF16, kind="ExternalInput")
    b_qkv = nc.dram_tensor("b_qkv", [128, 3, NH], F32, kind="ExternalInput")
    wo = nc.dram_tensor("wo", [D, NH, DOUT], BF16, kind="ExternalInput")
    ident = nc.dram_tensor("ident", [128, 128], BF16, kind="ExternalInput")
    bo = nc.dram_tensor("bo", [128, DOUT // 128], F32, kind="ExternalInput")
    outT = nc.dram_tensor("outT", [DOUT, S], BF16, kind="ExternalOutput")

    xq3 = [nc.sync, nc.gpsimd, nc.scalar]

    with tile.TileContext(nc) as tc:
        with (
            tc.tile_pool(name="const", bufs=1) as constp,
            tc.tile_pool(name="xin", bufs=17) as xp,
            tc.tile_pool(name="maskp", bufs=24) as mp,
            tc.tile_pool(name="pwork", bufs=3) as pw,
            tc.tile_pool(name="rwork", bufs=2) as rbp,
            tc.tile_pool(name="fout", bufs=4) as fop,
            tc.tile_pool(name="psum", bufs=4, space="PSUM") as psp,
            tc.tile_pool(name="dram", bufs=2, space="DRAM") as dramp,
        ):
            # ---- small constants + exp table preload ----
            ones_col = constp.tile([128, 1], BF16)
            nc.vector.memset(ones_col, 1.0)
            dummy = constp.tile([128, 1], BF16)
            # preload the exp table set so the first real exp doesn't pay it
            nc.scalar.activation(out=dummy, in_=ones_col, func=Exp, bias=0.0, scale=1.0)
            b_sb = constp.tile([128, 3, NH], F32)
            nc.gpsimd.dma_start(out=b_sb, in_=b_qkv[:])

            # ---- weights early on dedicated rings, then q/k chunk halves ----
            # v-chunk DMA *issues* are deferred until after the q/k passes so
            # no engine queue ever head-blocks on an x-slot WAR wait ahead of
            # the PSUM evacuations.
            w_sb = constp.tile([128, 3, KC, NH, D], BF16)
            nc.scalar.dma_start(out=w_sb[:, 0, :, :, :], in_=w_qkv[:, 0, :, :, :])
            ident_sb = constp.tile([128, 128], BF16)
            xts = []  # [t][k]
            for t in range(3):
                xts.append(
                    [xp.tile([128, S], BF16, tag="x", name=f"x{t}_{k}") for k in range(KC)]
                )
            qi = 0
            for t in range(2):
                for cb in range(2):
                    for k in range(KC):
                        xq3[qi % 3].dma_start(
                            out=xts[t][k][:, ts(cb, HQ)],
                            in_=xT[t, k * 128 : (k + 1) * 128, ts(cb, HQ)],
                        )
                        qi += 1
                    if t == 0 and cb == 0:
                        # q passA chunks are queued on every ring; later-needed
                        # weights ride behind them
                        nc.gpsimd.dma_start(
                            out=w_sb[:, 1, :, :, :], in_=w_qkv[:, 1, :, :, :]
                        )
                        nc.scalar.dma_start(out=ident_sb, in_=ident[:])
                    if t == 1 and cb == 0:
                        nc.gpsimd.dma_start(
                            out=w_sb[:, 2, :, :, :], in_=w_qkv[:, 2, :, :, :]
                        )

            mtile = {
                (half, oc): mp.tile([128, HQ], BF16, tag="mask", name=f"m{half}_{oc}")
                for half in range(2)
                for oc in range(OC)
            }
            bo_sb = constp.tile([128, DOUT // 128], F32)
            wo_sb = constp.tile([D, NH, DOUT], BF16)

            def issue_late_dmas():
                # called after the q/k projection passes are issued: v chunks,
                # then masks, then out-projection constants
                xq2 = [nc.sync, nc.gpsimd]
                n = 0
                for cb in range(2):
                    for k in range(KC):
                        xq2[n % 2].dma_start(
                            out=xts[2][k][:, ts(cb, HQ)],
                            in_=xT[2, k * 128 : (k + 1) * 128, ts(cb, HQ)],
                        )
                        n += 1
                # mask half0 on sync, half1 on gpsimd so each unit's r-chain
                # finds at least one drained ring
                for half in range(2):
                    for oc in range(OC):
                        xq2[half].dma_start(
                            out=mtile[(half, oc)],
                            in_=maskT[
                                oc * 128 : (oc + 1) * 128, half * HQ : (half + 1) * HQ
                            ],
                        )
                    if half == 0:
                        nc.gpsimd.dma_start(out=bo_sb, in_=bo[:])
                        nc.gpsimd.dma_start(out=wo_sb, in_=wo[:])

            qk_sb = constp.tile([128, 2, NH, S], BF16)
            vpt_sb = constp.tile([128, NH, S], BF16)
            vp_sb = constp.tile([128, OC, NH, D], BF16)
            outn_sb = constp.tile([128, NH, S], BF16)

            def evac_bias(n, dst, src, bias_ap):
                # alternate scalar/vector engines for PSUM evacuation
                if n % 2 == 0:
                    nc.scalar.activation(out=dst, in_=src, func=Ident, bias=bias_ap, scale=1.0)
                else:
                    nc.vector.tensor_scalar_add(out=dst, in0=src, scalar1=bias_ap)

            # ---- projections: two passes per tensor, chasing the DMA stream.
            # pass cb covers query-columns [cb*1024, cb*1024+1024) for BOTH
            # heads so each arriving x half-chunk is consumed immediately.
            def proj_pass(t, cb, dst):
                acc = {}
                for h in range(NH):
                    for i2 in range(2):
                        acc[(h, i2)] = psp.tile(
                            [128, NT], F32, tag="acc", name=f"acc{t}_{cb}_{h}_{i2}"
                        )
                for k in range(KC):
                    for h in range(NH):
                        for i2 in range(2):
                            nc.tensor.matmul(
                                acc[(h, i2)],
                                w_sb[:, t, k, h, :],
                                xts[t][k][:, ds(cb * HQ + i2 * NT, NT)],
                                start=(k == 0),
                                stop=(k == KC - 1),
                            )
                for n, h in enumerate(range(NH)):
                    for i2 in range(2):
                        evac_bias(
                            n * 2 + i2,
                            dst[h][:, ds(cb * HQ + i2 * NT, NT)],
                            acc[(h, i2)],
                            b_sb[:, t, h : h + 1],
                        )

            for t in range(2):
                for cb in range(2):
                    proj_pass(t, cb, [qk_sb[:, t, h, :] for h in range(NH)])

            issue_late_dmas()

            # ---- V projection + PE transpose to natural layout ----
            def v_transposes(cb):
                # key chunks oc in [cb*8, cb*8+8) for both heads
                for h in range(NH):
                    for oc in range(cb * 8, cb * 8 + 8):
                        tps = psp.tile(
                            [128, D], BF16, tag="s", bufs=2, name=f"tps{h}_{oc}"
                        )
                        nc.tensor.transpose(
                            tps, vpt_sb[:, h, ds(oc * 128, 128)], ident_sb
                        )
                        nc.vector.tensor_copy(vp_sb[:, oc, h, :], tps)

            # ---- attention units: (half, head), pipelined inner loop ----
            pending = []  # deferred DVE epilogue thunks from the previous unit

            def make_se(u, half, hh):
                i0 = half * HQ
                sps = {}
                pms = {}

                def scores(oc):
                    t_ = psp.tile([128, 2 * NT], F32, tag="s", bufs=2, name=f"sps{u}_{oc}")
                    for j in range(2):
                        nc.tensor.matmul(
                            t_[:, ts(j, NT)],
                            qk_sb[:, 1, hh, ds(oc * 128, 128)],
                            qk_sb[:, 0, hh, ds(i0 + j * NT, NT)],
                            start=True,
                            stop=True,
                        )
                    sps[oc] = t_

                def expmul(oc):
                    p = pw.tile([128, 2 * NT], BF16, tag="p", name=f"p{u}_{oc}")
                    nc.scalar.activation(
                        out=p, in_=sps.pop(oc), func=Exp, bias=0.0, scale=SCALE
                    )
                    pm = pw.tile([128, 2 * NT], BF16, tag="pm", name=f"pm{u}_{oc}")
                    nc.vector.tensor_mul(pm, p, mtile[(half, oc)])
                    pms[oc] = pm

                return scores, expmul, sps, pms


            # hoist u0's first scores + exps ahead of the V phase (PE is about
            # to wait on v-chunk DMAs; ACT is idle) — muls wait for mask in u0
            u0_state = make_se(0, 0, 0)
            u0_ps = {}
            u0_state[0](0)
            u0_state[0](1)
            for oc in (0, 1):
                p_ = pw.tile([128, 2 * NT], BF16, tag="p", name=f"p0h_{oc}")
                nc.scalar.activation(
                    out=p_, in_=u0_state[2].pop(oc), func=Exp, bias=0.0, scale=SCALE
                )
                u0_ps[oc] = p_

            proj_pass(2, 0, [vpt_sb[:, h, :] for h in range(NH)])
            proj_pass(2, 1, [vpt_sb[:, h, :] for h in range(NH)])
            v_transposes(0)
            v_transposes(1)

            def do_unit(u, half, hh):
                i0 = half * HQ
                outp = [
                    psp.tile([128, NT], F32, tag="acc", name=f"outp{u}_{j}")
                    for j in range(2)
                ]
                rp = psp.tile([128, NT], F32, tag="acc", name=f"rp{u}")
                if u != 0:
                    scores, expmul, sps, pms = make_se(u, half, hh)

                if u == 0:
                    # scores+exp for oc 0,1 were issued before the V phase;
                    # finish their mask-muls now (mask has long arrived)
                    scores, expmul, sps, pms = u0_state
                    for oc in (0, 1):
                        pm = pw.tile([128, 2 * NT], BF16, tag="pm", name=f"pm0_{oc}")
                        nc.vector.tensor_mul(pm, u0_ps.pop(oc), mtile[(0, oc)])
                        pms[oc] = pm
                else:
                    scores(0)
                    expmul(0)
                    scores(1)
                    expmul(1)
                for oc in range(OC):
                    # drain one deferred thunk from the previous unit's epilogue
                    if oc in (3, 5, 7) and pending:
                        pending.pop(0)()
                    if oc + 2 < OC:
                        scores(oc + 2)
                    pm = pms.pop(oc)
                    for j in range(2):
                        nc.tensor.matmul(
                            outp[j],
                            vp_sb[:, oc, hh, :],
                            pm[:, ts(j, NT)],
                            start=(oc == 0),
                            stop=(oc == OC - 1),
                        )
                    for j in range(2):
                        nc.tensor.matmul(
                            rp[32 * j : 32 * j + 1, :],
                            ones_col,
                            pm[:, ts(j, NT)],
                            start=(oc == 0),
                            stop=(oc == OC - 1),
                        )
                    if oc + 2 < OC:
                        expmul(oc + 2)

                # epilogue: evacuate rowsums first (critical path), then outp;
                # two per-j r-chains run in parallel on separate queues.
                # reciprocal + normalize deferred (returned as thunks)
                rq = [nc.gpsimd, nc.sync]
                r2 = rbp.tile([33, NT], F32, tag="r2", name=f"r2_{u}")
                for j in range(2):
                    nc.vector.tensor_copy(
                        r2[32 * j : 32 * j + 1, :], rp[32 * j : 32 * j + 1, :]
                    )
                rd = dramp.tile([2, NT], F32, tag="rd", name=f"rd{u}")
                for j in range(2):
                    rq[j].dma_start(out=rd[j : j + 1, :], in_=r2[32 * j : 32 * j + 1, :])
                rsegs = [
                    rbp.tile([64, 8], F32, tag=f"rseg{j}", name=f"rseg{u}_{j}")
                    for j in range(2)
                ]
                for j in range(2):
                    rq[j].dma_start(
                        out=rsegs[j],
                        in_=rd[j : j + 1, :]
                        .rearrange("a b -> (a b)")
                        .rearrange("(p c) -> p c", p=64),
                    )
                osb = rbp.tile([128, 2 * NT], F32, tag="osb", name=f"osb{u}")
                nc.scalar.copy(osb[:, ts(0, NT)], outp[0])
                nc.vector.tensor_copy(osb[:, ts(1, NT)], outp[1])
                rsegs2 = [
                    rbp.tile([64, 8], F32, tag=f"rseg2_{j}", name=f"rseg2_{u}_{j}")
                    for j in range(2)
                ]
                rd2 = dramp.tile([2, NT], F32, tag="rd2", name=f"rd2_{u}")
                rbcs = [
                    rbp.tile([128, NT], F32, tag="rbc", name=f"rbc{u}_{j}")
                    for j in range(2)
                ]

                def t_recip(j):
                    nc.vector.reciprocal(rsegs2[j], rsegs[j])
                    rq[j].dma_start(
                        out=rd2[j : j + 1, :]
                        .rearrange("a b -> (a b)")
                        .rearrange("(p c) -> p c", p=64),
                        in_=rsegs2[j],
                    )
                    rq[j].dma_start(
                        out=rbcs[j], in_=rd2[j : j + 1, :].to_broadcast([128, NT])
                    )

                def t_norm(j):
                    # normalize on the (idle) gpsimd engine: all-SBUF operands,
                    # keeps the vector queue free for the next unit's mask-muls
                    nc.gpsimd.tensor_mul(
                        outn_sb[:, hh, ds(i0 + j * NT, NT)], osb[:, ts(j, NT)], rbcs[j]
                    )

                return [
                    lambda: t_recip(0),
                    lambda: (t_recip(1), t_norm(0)),
                    lambda: t_norm(1),
                ]

            def window(half, lo=0, hi=16):
                # out-projection tiles [lo, hi) for one query half (n = itl*8+dc)
                i0 = half * HQ
                outq = [nc.sync, nc.gpsimd, nc.scalar]
                for n in range(lo, hi):
                    itl, dc = n // 8, n % 8
                    # alternate PSUM tags for a deeper evacuation pipeline
                    if n % 2 == 0:
                        facc = psp.tile(
                            [128, NT], F32, tag="s", bufs=2, name=f"facc{half}_{itl}_{dc}"
                        )
                    else:
                        facc = psp.tile(
                            [128, NT], F32, tag="acc", name=f"facc{half}_{itl}_{dc}"
                        )
                    for hh in range(NH):
                        nc.tensor.matmul(
                            facc,
                            wo_sb[:, hh, ds(dc * 128, 128)],
                            outn_sb[:, hh, ds(i0 + itl * NT, NT)],
                            start=(hh == 0),
                            stop=(hh == NH - 1),
                        )
                    fsb = fop.tile([128, NT], BF16, tag="f", name=f"f{half}_{itl}_{dc}")
                    evac_bias(n, fsb, facc, bo_sb[:, dc : dc + 1])
                    outq[n % 3].dma_start(
                        out=outT[ds(dc * 128, 128), ds(i0 + itl * NT, NT)], in_=fsb
                    )

            # order: u0, u1, u2, most of winA(half0), u3, rest of winA, winB.
            # winA placed after u2 so u1's r-chain latency hides inside u2;
            # winA's last tiles run after u3 to cover u3's r-chain latency.
            pending.extend(do_unit(0, 0, 0))
            pending.extend(do_unit(1, 0, 1))
            pending.extend(do_unit(2, 1, 0))
            window(0, 0, 8)
            pending.extend(do_unit(3, 1, 1))
            # drain u3's r-chain immediately (recip on DVE, norms on gpsimd)
            # so winA's remaining tiles execute while it resolves
            for th in pending:
                th()
            pending.clear()
            window(0, 8, 16)
            window(1)

    return nc


_NC_CACHE = None


def _get_nc():
    global _NC_CACHE
    if _NC_CACHE is None:
        nc = build()
        nc.compile()
        _NC_CACHE = nc
    return _NC_CACHE


def make_in_maps(q, k, v, mask, Wq, bq, Wk, bk, Wv, bv, Wo, bo):
    bf = ml_dtypes.bfloat16
    q = np.asarray(q, np.float32)
    k = np.asarray(k, np.float32)
    v = np.asarray(v, np.float32)
    mask = np.asarray(mask)
    Ws = [np.asarray(w, np.float32) for w in (Wq, Wk, Wv)]
    bs = [np.asarray(b_, np.float32) for b_ in (bq, bk, bv)]
    Wo = np.asarray(Wo, np.float32)
    bo = np.asarray(bo, np.float32)

    xTb, maskTb = [], []
    for b in range(B):
        xTb.append(
            np.ascontiguousarray(np.stack([q[b].T, k[b].T, v[b].T]).astype(bf))
        )
        maskTb.append(
            np.ascontiguousarray(mask[b].T.astype(np.float32)).astype(bf)
        )
    # W[dm, dout] with head h owning columns d*H+h; reshape for tile slicing:
    # Wr[t][kc, p, d, h] = W[kc*128+p, d*H+h]
    Wr = [W.reshape(KC, 128, D, H) for W in Ws]
    br = [b_.reshape(D, H) for b_ in bs]

    ident = np.eye(128, dtype=np.float32).astype(bf)

    in_maps = []
    for c in range(8):
        b = c // 4
        h0 = NH * (c % 4)
        w_core = np.empty((128, 3, KC, NH, D), np.float32)
        for t in range(3):
            for hi in range(NH):
                w_core[:, t, :, hi, :] = Wr[t][:, :, :, h0 + hi].transpose(1, 0, 2)
        b_core = np.empty((128, 3, NH), np.float32)
        for t in range(3):
            for hi in range(NH):
                b_core[:, t, hi] = br[t][:, h0 + hi]
        wo_core = np.stack([Wo[h0 + hi :: H, :] for hi in range(NH)], axis=1)
        bo_core = bo if c % 4 == 0 else np.zeros_like(bo)
        in_maps.append(
            {
                "xT": xTb[b],
                "ident": ident,
                "maskT": maskTb[b],
                "w_qkv": np.ascontiguousarray(w_core).astype(bf),
                "b_qkv": np.ascontiguousarray(b_core),
                "wo": np.ascontiguousarray(wo_core).astype(bf),
                "bo": np.ascontiguousarray(bo_core.reshape(DOUT // 128, 128).T),
            }
        )
    return in_maps


def unshard(results):
    out = np.zeros((B, DOUT, S), np.float32)
    for c in range(8):
        out[c // 4] += np.asarray(results[c]["outT"], np.float32)
    return np.ascontiguousarray(out.transpose(0, 2, 1))


def kernel(**inputs):
    in_maps = make_in_maps(**inputs)
    nc = _get_nc()
    res = run_bass_kernel_spmd(nc, in_maps, core_ids=list(range(8)))
    return unshard(res.results)
